# revision 8
# baseline (speedup 1.0000x reference)
"""Trainium2 8-core kernel for nn_CAT_81269371175150 (GNN message passing).

Math (see reference):
  gcn(x)   = selu(A_gn @ (x @ W1^T))            for features and aug_features
  S        = softmax_K(gcn1 @ Wt^T)
  loss     = spectral(S, A) + cluster(S) + 0.5 * con(gcn1, gcn2)

Strategy:
  * Nodes sharded row-wise across 8 cores; edge list bucketed by dest shard.
  * h = X @ W1^T computed per-shard (bf16), AllGather'd to every core's HBM.
  * SpMM A @ h done as: dma_gather of h[col] rows (bf16) + one-hot matmul
    segment-sum into PSUM per 128-row destination block.
  * Second GCN fuses A @ S into the same matmul pass (combined gather table
    rows [h2 | S | pad]); the one-hot carries gn_vals, so gathered S rows are
    pre-scaled by 1/gn.
  * log-softmax over nodes and all final reductions are returned as per-core
    partials; the (tiny) final combine happens on host.
"""

import math
import numpy as np
import ml_dtypes

import concourse.bacc as bacc
import concourse.mybir as mybir
import concourse.tile as tile
from concourse import bass_utils
from concourse.masks import make_identity

P = 128
NC = 8

# full-size problem constants
FULL = dict(N=50000, F=500, D=256, K=16)

SELU_L = 1.0507009873554805
SELU_A = 1.6732632423543772
SELU_LA = SELU_L * SELU_A
LN_SELU_LA = math.log(SELU_LA)

CLUSTER_REG = 1.0
CON_REG = 0.5

bf16 = mybir.dt.bfloat16
f32 = mybir.dt.float32
i16 = mybir.dt.int16


def cdiv(a, b):
    return -(-a // b)


# --------------------------------------------------------------------------
# host-side preprocessing
# --------------------------------------------------------------------------

def prep(features, aug_features, graph_row, graph_col, gn_vals, W1, Wt, cfg):
    N, F, D, K = cfg["N"], cfg["F"], cfg["D"], cfg["K"]
    NSH = N // NC
    NB = cdiv(NSH, P)
    HALF = N // 2
    DT = D // P
    W2 = D + P               # combined table row width (bf16 elems)

    row = np.asarray(graph_row).astype(np.int64)
    col = np.asarray(graph_col).astype(np.int64)
    gn = np.asarray(gn_vals).astype(np.float64)
    gn = np.maximum(gn, 1e-12)

    core = row // NSH
    # per (core, block, half) counts
    per_core = []
    cnts = np.zeros((NC, NB, 2), dtype=np.int64)
    for c in range(NC):
        m = core == c
        r = row[m] - c * NSH
        cl = col[m]
        g = gn[m]
        b = r // P
        h = cl // HALF
        order = np.lexsort((cl, h, b))
        r, cl, g, b, h = r[order], cl[order], g[order], b[order], h[order]
        key = b * 2 + h
        cnt = np.bincount(key, minlength=NB * 2).reshape(NB, 2)
        cnts[c] = cnt
        per_core.append((r, cl, g, b, h, key))

    CBH = np.ceil(cnts.max(axis=0) / P).astype(np.int64)       # [NB, 2]
    nch_b = CBH[:, 0] + CBH[:, 1]                               # chunks per block
    NCHT = int(nch_b.sum())
    # stream bases (in chunks) per half
    strm_base = np.zeros((2, NB), dtype=np.int64)
    for h in range(2):
        strm_base[h] = np.concatenate([[0], np.cumsum(CBH[:, h])[:-1]])
    Lh = [int(CBH[:, h].sum()) * P for h in range(2)]            # idx entries per half
    ohbase = np.concatenate([[0], np.cumsum(nch_b)[:-1]])        # oh col base per block

    deg = np.bincount(col, minlength=N).astype(np.float64)

    X = np.asarray(features)[0]
    Xa = np.asarray(aug_features)[0]
    XT = np.ascontiguousarray(X.T).astype(ml_dtypes.bfloat16)    # [F, N]
    XTa = np.ascontiguousarray(Xa.T).astype(ml_dtypes.bfloat16)
    W1T = np.ascontiguousarray(np.asarray(W1).T).astype(ml_dtypes.bfloat16)  # [F, D]
    WtT = np.ascontiguousarray(np.asarray(Wt).T).astype(ml_dtypes.bfloat16)  # [D, K]

    rowmask = np.zeros((P, NB), dtype=np.float32)
    for b in range(NB):
        rows = min(P, NSH - b * P)
        rowmask[:rows, b] = 1.0

    def wrap_idx(a):
        # [L] -> [128, L/16]: element i at [i%16, i//16], replicated x8
        L = a.shape[0]
        assert L % 16 == 0
        w = a.reshape(L // 16, 16).T
        return np.ascontiguousarray(np.tile(w, (8, 1)))

    in_maps = []
    for c in range(NC):
        r, cl, g, b, h, key = per_core[c]
        cnt = cnts[c]
        # rank of each edge within its (b,h) run
        run_start = np.zeros(NB * 2, dtype=np.int64)
        flat = cnt.reshape(-1)
        run_start[1:] = np.cumsum(flat)[:-1]
        rank = np.arange(len(r)) - run_start[key]

        lane = rank % P
        j = rank // P

        # idx streams
        idx_streams = []
        for hh in range(2):
            arr = np.zeros(Lh[hh], dtype=np.int16)
            m = h == hh
            off = (strm_base[hh][b[m]] + j[m]) * P + lane[m]
            arr[off] = (cl[m] - hh * HALF).astype(np.int16)
            idx_streams.append(wrap_idx(arr))

        # one-hot stream [128, NCHT, 128] and rgn [128, NCHT]
        oh = np.zeros((P, NCHT, P), dtype=ml_dtypes.bfloat16)
        rgn = np.zeros((P, NCHT), dtype=np.float32)
        ohcol = ohbase[b] + h * CBH[b, 0] + j
        dest = r - b * P
        oh[lane, ohcol, dest] = g.astype(ml_dtypes.bfloat16)
        rgn[lane, ohcol] = (1.0 / g).astype(np.float32)

        # degree, blocked [P, NB]
        dpad = np.zeros(NB * P, dtype=np.float32)
        dpad[:NSH] = deg[c * NSH:(c + 1) * NSH]
        deg_blk = np.ascontiguousarray(dpad.reshape(NB, P).T)

        in_maps.append({
            "xt": np.ascontiguousarray(XT[:, c * NSH:(c + 1) * NSH]),
            "xta": np.ascontiguousarray(XTa[:, c * NSH:(c + 1) * NSH]),
            "w1t": W1T,
            "wtt": WtT,
            "deg": deg_blk,
            "rowmask": rowmask,
            "oh": oh,
            "rgn": rgn,
            "idx0": idx_streams[0],
            "idx1": idx_streams[1],
        })

    meta = dict(
        N=N, F=F, D=D, K=K, NSH=NSH, NB=NB, HALF=HALF, DT=DT, W2=W2,
        CBH=tuple(map(tuple, CBH.tolist())), NCHT=NCHT,
        strm_base=tuple(map(tuple, strm_base.tolist())),
        Lh=tuple(Lh), ohbase=tuple(ohbase.tolist()),
        FT=cdiv(F, P),
    )
    return in_maps, meta


# --------------------------------------------------------------------------
# device program
# --------------------------------------------------------------------------

def build(meta, debug=False):
    N, F, D, K = meta["N"], meta["F"], meta["D"], meta["K"]
    NSH, NB, HALF, DT, W2 = meta["NSH"], meta["NB"], meta["HALF"], meta["DT"], meta["W2"]
    CBH = meta["CBH"]
    NCHT = meta["NCHT"]
    strm_base = meta["strm_base"]
    Lh = meta["Lh"]
    ohbase = meta["ohbase"]
    FT = meta["FT"]
    GB = 2                      # blocks per gather group
    VLEN = 2 * D + 3 * K

    nc = bacc.Bacc("TRN2", target_bir_lowering=False, debug=debug,
                   num_devices=NC)

    xt = nc.dram_tensor("xt", [F, NSH], bf16, kind="ExternalInput")
    xta = nc.dram_tensor("xta", [F, NSH], bf16, kind="ExternalInput")
    w1t = nc.dram_tensor("w1t", [F, D], bf16, kind="ExternalInput")
    wtt = nc.dram_tensor("wtt", [D, K], bf16, kind="ExternalInput")
    deg = nc.dram_tensor("deg", [P, NB], f32, kind="ExternalInput")
    rowmask = nc.dram_tensor("rowmask", [P, NB], f32, kind="ExternalInput")
    oh = nc.dram_tensor("oh", [P, NCHT, P], bf16, kind="ExternalInput")
    rgn_d = nc.dram_tensor("rgn", [P, NCHT], f32, kind="ExternalInput")
    idx_d = [nc.dram_tensor(f"idx{h}", [P, Lh[h] // 16], i16, kind="ExternalInput")
             for h in range(2)]

    stats_p_d = nc.dram_tensor("stats_p", [P, 2 * DT], f32, kind="ExternalOutput")
    stats_v_d = nc.dram_tensor("stats_v", [1, VLEN], f32, kind="ExternalOutput")

    max_nch_b = max(CBH[b][0] + CBH[b][1] for b in range(NB))
    # gather group sizes (chunks) per half
    grp = {h: [] for h in range(2)}   # list of (start_chunk, nchunks)
    for h in range(2):
        for g0 in range(0, NB, GB):
            s = strm_base[h][g0]
            n = sum(CBH[b][h] for b in range(g0, min(g0 + GB, NB)))
            grp[h].append((s, n))
    max_grp = max((n for h in range(2) for (_, n) in grp[h]), default=1)

    big_bytes = max(NSH * 2, max_grp * W2 * 2, max_grp * D * 2)

    with tile.TileContext(nc) as tc:
        with (
            tc.tile_pool(name="big", bufs=4) as bigp,
            tc.tile_pool(name="ohp", bufs=3) as ohp,
            tc.tile_pool(name="persist", bufs=1) as persist,
            tc.tile_pool(name="stage", bufs=3) as stagep,
            tc.tile_pool(name="tmp", bufs=2) as tmpp,
            tc.tile_pool(name="small", bufs=4) as smallp,
            tc.tile_pool(name="svp", bufs=1) as svp,
            tc.tile_pool(name="pa", bufs=3, space="PSUM") as pa,
            tc.tile_pool(name="pb", bufs=4, space="PSUM") as pb,
            tc.tile_pool(name="dram", bufs=1, space="DRAM") as dramp,
        ):
            # ---- constants / resident tensors
            ident = persist.tile([P, P], f32)
            make_identity(nc, ident[:])
            w1t_t = persist.tile([P, FT, D], bf16)
            for t in range(FT):
                fr = min(P, F - t * P)
                nc.sync.dma_start(w1t_t[:fr, t, :], w1t[t * P:t * P + fr, :])
            wtt_t = persist.tile([P, DT, K], bf16)
            for t in range(DT):
                nc.sync.dma_start(wtt_t[:, t, :], wtt[t * P:(t + 1) * P, :])
            deg_t = persist.tile([P, NB], f32)
            nc.sync.dma_start(deg_t[:], deg[:])
            mask_t = persist.tile([P, NB], f32)
            nc.sync.dma_start(mask_t[:], rowmask[:])
            rgn_t = persist.tile([P, NCHT], f32)
            nc.sync.dma_start(rgn_t[:], rgn_d[:])
            idx_t = []
            for h in range(2):
                it = persist.tile([P, Lh[h] // 16], i16, tag=f"idx{h}")
                nc.sync.dma_start(it[:], idx_d[h][:])
                idx_t.append(it)

            ln_la = persist.tile([P, 1], f32, tag="lnla")
            nc.vector.memset(ln_la[:], LN_SELU_LA)

            gcn1 = persist.tile([P, NB, D], f32, tag="gcn1")
            gcn1T = persist.tile([P, DT, NB * P], bf16, tag="gcn1T")
            Sv = persist.tile([P, NB, K], f32, tag="Sv")
            accs = persist.tile([P, VLEN], f32, tag="accs")
            nc.vector.memset(accs[:], 0.0)
            stats_p = persist.tile([P, 2 * DT], f32, tag="statsp")

            # ---- DRAM collective buffers
            cc1_in = dramp.tile([NSH, D], bf16)
            cc1_out = dramp.tile([N, D], bf16, addr_space="Shared")
            cc2_in = dramp.tile([NSH, W2], bf16)
            cc2_out = dramp.tile([N, W2], bf16, addr_space="Shared")

            # ================= phase A: h1 = X @ W1^T =================
            def dense_h(src_dram, cc_in, with_s):
                xt_tiles = []
                for t in range(FT):
                    fr = min(P, F - t * P)
                    xx = bigp.tile([P, NSH], bf16, tag="big")
                    nc.sync.dma_start(xx[:fr, :], src_dram[t * P:t * P + fr, :])
                    xt_tiles.append(xx)
                for b in range(NB):
                    rows = min(P, NSH - b * P)
                    pt = pa.tile([P, D], f32, space="PSUM", tag="pa")
                    for t in range(FT):
                        fr = min(P, F - t * P)
                        nc.tensor.matmul(
                            pt[:rows, :],
                            lhsT=xt_tiles[t][:fr, b * P:b * P + rows],
                            rhs=w1t_t[:fr, t, :],
                            start=(t == 0), stop=(t == FT - 1),
                        )
                    if with_s:
                        st = stagep.tile([P, W2], bf16, tag="stage2")
                        nc.vector.tensor_copy(st[:rows, 0:D], pt[:rows, :])
                        nc.vector.tensor_copy(st[:rows, D:D + K], Sv[:rows, b, :])
                        nc.vector.memset(st[:rows, D + K:W2], 0.0)
                        nc.sync.dma_start(cc_in[b * P:b * P + rows, :], st[:rows, :])
                    else:
                        st = stagep.tile([P, D], bf16, tag="stage1")
                        nc.vector.tensor_copy(st[:rows, :], pt[:rows, :])
                        nc.sync.dma_start(cc_in[b * P:b * P + rows, :], st[:rows, :])

            dense_h(xt, cc1_in, with_s=False)
            nc.gpsimd.collective_compute(
                "AllGather", mybir.AluOpType.bypass,
                replica_groups=[list(range(NC))],
                ins=[cc1_in[:]], outs=[cc1_out[:]],
            )

            # ================= SpMM pass helper =================
            def spmm_pass(cc_out, W, gcn2):
                """Iterate blocks; gather h rows; one-hot matmul; epilogue cb."""
                gtile = {}     # (h, g) -> (tile, start_chunk)

                def get_gtile(h, g):
                    if (h, g) not in gtile:
                        s, n = grp[h][g]
                        gt = bigp.tile([P, max_grp, W], bf16, tag="big")
                        if n > 0:
                            nidx = n * P
                            view = cc_out[h * HALF:(h + 1) * HALF, :]
                            nc.gpsimd.dma_gather(
                                gt[:, 0:n, :], view, idx_t[h][:, s * 8:(s + n) * 8],
                                num_idxs=nidx, num_idxs_reg=nidx, elem_size=W,
                                single_packet=False,
                            )
                        gtile[(h, g)] = (gt, s)
                    return gtile[(h, g)]

                for b in range(NB):
                    nch = CBH[b][0] + CBH[b][1]
                    oht = ohp.tile([P, max_nch_b, P], bf16, tag="oh")
                    nc.sync.dma_start(
                        oht[:, 0:nch, :],
                        oh[:, ohbase[b]:ohbase[b] + nch, :])
                    pt = pa.tile([P, D], f32, space="PSUM", tag="pa")
                    if gcn2:
                        pt2 = pb.tile([P, P], f32, space="PSUM", tag="pb")
                    nmm = 0
                    tot = sum(CBH[b])
                    if tot == 0:
                        nc.vector.memset(pt[:], 0.0)
                        if gcn2:
                            nc.vector.memset(pt2[:], 0.0)
                    for h in range(2):
                        g = b // GB
                        for j in range(CBH[b][h]):
                            gt, s = get_gtile(h, g)
                            c = strm_base[h][b] + j - s
                            lhs = oht[:, h * CBH[b][0] + j, :]
                            nc.tensor.matmul(
                                pt[:], lhsT=lhs, rhs=gt[:, c, 0:D],
                                start=(nmm == 0), stop=(nmm == tot - 1))
                            if gcn2:
                                kcol = ohbase[b] + h * CBH[b][0] + j
                                rs = stagep.tile([P, P], bf16, tag="sresc")
                                nc.vector.tensor_scalar(
                                    rs[:], gt[:, c, D:D + P],
                                    rgn_t[:, kcol:kcol + 1], None,
                                    mybir.AluOpType.mult)
                                nc.tensor.matmul(
                                    pt2[:], lhsT=lhs, rhs=rs[:],
                                    start=(nmm == 0), stop=(nmm == tot - 1))
                            nmm += 1
                    # release finished gather tiles happens via pool rotation
                    yield b, pt, (pt2 if gcn2 else None)

            def selu_into(dst_ap, psum_ap):
                """dst = selu(psum)  (f32)"""
                e2 = tmpp.tile([P, D], f32, tag="tmpd")
                nc.scalar.activation(e2[:], psum_ap, mybir.ActivationFunctionType.Exp,
                                     bias=ln_la[:])
                nc.vector.tensor_scalar(e2[:], e2[:], SELU_LA, 0.0,
                                        mybir.AluOpType.subtract,
                                        mybir.AluOpType.min)
                r = tmpp.tile([P, D], f32, tag="tmpd2")
                nc.scalar.activation(r[:], psum_ap, mybir.ActivationFunctionType.Relu,
                                     scale=SELU_L)
                nc.vector.tensor_tensor(dst_ap, e2[:], r[:], mybir.AluOpType.add)

            # ================= phase B: GCN1 =================
            for b, pt, _ in spmm_pass(cc1_out, D, gcn2=False):
                g1b = gcn1[:, b, :]
                selu_into(g1b, pt[:])
                # transpose into gcn1T
                for t in range(DT):
                    ptr = pb.tile([P, P], f32, space="PSUM", tag="pb")
                    nc.tensor.transpose(ptr[:], g1b[:, t * P:(t + 1) * P], ident[:])
                    nc.vector.tensor_copy(gcn1T[:, t, b * P:(b + 1) * P], ptr[:])
                # logits -> softmax -> S
                pl = pb.tile([P, K], f32, space="PSUM", tag="pb")
                for t in range(DT):
                    nc.tensor.matmul(pl[:], lhsT=gcn1T[:, t, b * P:(b + 1) * P],
                                     rhs=wtt_t[:, t, :],
                                     start=(t == 0), stop=(t == DT - 1))
                nmx = smallp.tile([P, 1], f32, tag="nmx")
                nc.vector.reduce_max(nmx[:], pl[:], axis=mybir.AxisListType.X,
                                     negate=True)
                ex = smallp.tile([P, K], f32, tag="ex")
                nc.scalar.activation(ex[:], pl[:], mybir.ActivationFunctionType.Exp,
                                     bias=nmx[:])
                sm = smallp.tile([P, 1], f32, tag="sm")
                nc.vector.reduce_sum(sm[:], ex[:], axis=mybir.AxisListType.X)
                rc = smallp.tile([P, 1], f32, tag="rc")
                nc.vector.reciprocal(rc[:], sm[:])
                Sb = Sv[:, b, :]
                nc.vector.tensor_scalar(Sb, ex[:], rc[:], mask_t[:, b:b + 1],
                                        mybir.AluOpType.mult,
                                        mybir.AluOpType.mult)
                # nl += S * deg ; clsz += S
                t1 = smallp.tile([P, K], f32, tag="t1")
                nc.vector.tensor_scalar(t1[:], Sb, deg_t[:, b:b + 1], None,
                                        mybir.AluOpType.mult)
                nc.vector.tensor_tensor(accs[:, 2 * D:2 * D + K],
                                        accs[:, 2 * D:2 * D + K], t1[:],
                                        mybir.AluOpType.add)
                nc.vector.tensor_tensor(accs[:, 2 * D + K:2 * D + 2 * K],
                                        accs[:, 2 * D + K:2 * D + 2 * K], Sb,
                                        mybir.AluOpType.add)

            # ================= phase C: h2 + combined table =================
            dense_h(xta, cc2_in, with_s=True)
            nc.gpsimd.collective_compute(
                "AllGather", mybir.AluOpType.bypass,
                replica_groups=[list(range(NC))],
                ins=[cc2_in[:]], outs=[cc2_out[:]],
            )

            # ================= phase D: GCN2 fused =================
            for b, pt, pt2 in spmm_pass(cc2_out, W2, gcn2=True):
                aug = tmpp.tile([P, D], f32, tag="aug")
                selu_into(aug[:], pt[:])
                nc.vector.tensor_tensor(accs[:, 0:D], accs[:, 0:D], aug[:],
                                        mybir.AluOpType.add)
                pr = tmpp.tile([P, D], f32, tag="pr")
                nc.vector.tensor_tensor(pr[:], aug[:], gcn1[:, b, :],
                                        mybir.AluOpType.mult)
                nc.vector.tensor_tensor(accs[:, D:2 * D], accs[:, D:2 * D], pr[:],
                                        mybir.AluOpType.add)
                tk = smallp.tile([P, K], f32, tag="tk")
                nc.vector.tensor_tensor(tk[:], Sv[:, b, :], pt2[:, 0:K],
                                        mybir.AluOpType.mult)
                nc.vector.tensor_tensor(accs[:, 2 * D + 2 * K:2 * D + 3 * K],
                                        accs[:, 2 * D + 2 * K:2 * D + 3 * K],
                                        tk[:], mybir.AluOpType.add)

            # ================= phase E: stats =================
            for t in range(DT):
                nmt = smallp.tile([P, 1], f32, tag="nmt")
                nc.vector.reduce_max(nmt[:], gcn1T[:, t, 0:NSH],
                                     axis=mybir.AxisListType.X, negate=True)
                nc.vector.tensor_scalar(stats_p[:, t:t + 1], nmt[:], -1.0, None,
                                        mybir.AluOpType.mult)
                exb = bigp.tile([P, NSH], bf16, tag="big")
                nc.scalar.activation(exb[:, 0:NSH], gcn1T[:, t, 0:NSH],
                                     mybir.ActivationFunctionType.Exp,
                                     bias=nmt[:],
                                     accum_out=stats_p[:, DT + t:DT + t + 1])
            sv = svp.tile([1, VLEN], f32, tag="sv")
            nc.gpsimd.tensor_reduce(sv[:], accs[:], axis=mybir.AxisListType.C,
                                    op=mybir.AluOpType.add)
            nc.sync.dma_start(stats_v_d[:], sv[:])
            nc.sync.dma_start(stats_p_d[:], stats_p[:])

    nc.compile()
    return nc


# --------------------------------------------------------------------------
# host-side combine of per-core partials
# --------------------------------------------------------------------------

def combine(results, cfg, n_edges):
    N, D, K = cfg["N"], cfg["D"], cfg["K"]
    DT = D // P
    m = []          # [NC, D] local col maxes
    s = []          # [NC, D] local sumexp(x - m_local)
    colsum_aug = np.zeros(D)
    dot = 0.0
    nl = np.zeros(K)
    clsz = np.zeros(K)
    trace_gp = 0.0
    for c in range(NC):
        sp = np.asarray(results[c]["stats_p"], dtype=np.float64)
        svv = np.asarray(results[c]["stats_v"], dtype=np.float64).reshape(-1)
        mc = np.concatenate([sp[:, t] for t in range(DT)])
        sc = np.concatenate([sp[:, DT + t] for t in range(DT)])
        m.append(mc)
        s.append(sc)
        colsum_aug += svv[0:D]
        dot += svv[D:2 * D].sum()
        nl += svv[2 * D:2 * D + K]
        clsz += svv[2 * D + K:2 * D + 2 * K]
        trace_gp += svv[2 * D + 2 * K:2 * D + 3 * K].sum()
    m = np.stack(m)
    s = np.stack(s)
    M = m.max(axis=0)
    Sg = (np.exp(m - M) * s).sum(axis=0)
    logZ = M + np.log(Sg)

    E = float(n_edges)
    spectral = -(trace_gp - (nl ** 2).sum() / (2.0 * E)) / (2.0 * E)
    cluster = (np.linalg.norm(clsz) / N * math.sqrt(K) - 1.0) * CLUSTER_REG
    con = -(dot - (logZ * colsum_aug).sum()) / D
    return spectral + cluster + CON_REG * con


# --------------------------------------------------------------------------
# entry point
# --------------------------------------------------------------------------

_BUILD_CACHE = {}


def kernel(features, aug_features, graph_row, graph_col, graph_vals, gn_vals,
           lbl, dense_graph, W1, b1, Wt, bt, _cfg=None, _trace=False):
    cfg = _cfg or FULL
    in_maps, meta = prep(features, aug_features, graph_row, graph_col,
                         gn_vals, W1, Wt, cfg)
    key = tuple(sorted((k, v) for k, v in meta.items()))
    if key not in _BUILD_CACHE:
        _BUILD_CACHE[key] = build(meta)
    nc = _BUILD_CACHE[key]
    res = bass_utils.run_bass_kernel_spmd(nc, in_maps, core_ids=list(range(NC)),
                                          trace=_trace)
    loss = combine(res.results, cfg, graph_row.shape[0])
    out = np.array(loss, dtype=np.float32)
    if _trace:
        return out, res
    return out


# revision 10
# speedup vs baseline: 2.1304x; 2.1304x over previous
"""Trainium2 8-core kernel for nn_CAT_81269371175150 (GNN message passing).

Math (see reference):
  gcn(x)   = selu(A_gn @ (x @ W1^T))            for features and aug_features
  S        = softmax_K(gcn1 @ Wt^T)
  loss     = spectral(S, A) + cluster(S) + 0.5 * con(gcn1, gcn2)

Strategy (v2):
  * Nodes sharded row-wise across 8 cores; edge list bucketed by destination
    shard, sorted by (dest block, src half, src col), padded to fixed
    chunk counts so all cores run one SPMD program.
  * h1 = X @ W1^T and h2 = Xa @ W1^T computed per-shard (bf16), packed as
    combined 1KB table rows [h1 | h2], AllGather'd to every core's HBM.
  * Single SpMM pass: dma_gather of table[col] rows (1KB) + one matmul per
    128-edge chunk (one-hot lhsT with gn values folded in) accumulating
    [A@h1 | A@h2] into one PSUM bank per destination block.
  * selu runs mostly on ScalarE; S = softmax_K is computed per block and
    written out; log-softmax stats + con-loss partials accumulate on-chip.
  * Host finishes the tiny reductions: trace(S^T A S), nl, cluster sizes,
    log-softmax renormalization across cores, final scalar.
"""

import math
import numpy as np
import ml_dtypes

import concourse.bacc as bacc
import concourse.mybir as mybir
import concourse.tile as tile
from concourse import bass_utils
from concourse.masks import make_identity

P = 128
NC = 8

# full-size problem constants
FULL = dict(N=50000, F=500, D=256, K=16)

SELU_L = 1.0507009873554805
SELU_A = 1.6732632423543772
SELU_LA = SELU_L * SELU_A
LN_SELU_LA = math.log(SELU_LA)

CLUSTER_REG = 1.0
CON_REG = 0.5

bf16 = mybir.dt.bfloat16
f32 = mybir.dt.float32
i16 = mybir.dt.int16


def cdiv(a, b):
    return -(-a // b)


# --------------------------------------------------------------------------
# host-side preprocessing
# --------------------------------------------------------------------------

def prep(features, aug_features, graph_row, graph_col, gn_vals, W1, Wt, cfg):
    N, F, D, K = cfg["N"], cfg["F"], cfg["D"], cfg["K"]
    NSH = N // NC
    NB = cdiv(NSH, P)
    HALF = N // 2

    row = np.asarray(graph_row).astype(np.int64)
    col = np.asarray(graph_col).astype(np.int64)
    gn = np.asarray(gn_vals).astype(np.float64)

    core = row // NSH
    per_core = []
    cnts = np.zeros((NC, NB, 2), dtype=np.int64)
    for c in range(NC):
        m = core == c
        r = row[m] - c * NSH
        cl = col[m]
        g = gn[m]
        b = r // P
        h = cl // HALF
        order = np.lexsort((cl, h, b))
        r, cl, g, b, h = r[order], cl[order], g[order], b[order], h[order]
        key = b * 2 + h
        cnt = np.bincount(key, minlength=NB * 2).reshape(NB, 2)
        cnts[c] = cnt
        per_core.append((r, cl, g, b, h, key))

    CBH = np.ceil(cnts.max(axis=0) / P).astype(np.int64)        # [NB, 2]
    nch_b = CBH[:, 0] + CBH[:, 1]
    NCHT = int(nch_b.sum())
    strm_base = np.zeros((2, NB), dtype=np.int64)
    for h in range(2):
        strm_base[h] = np.concatenate([[0], np.cumsum(CBH[:, h])[:-1]])
    Lh = [int(CBH[:, h].sum()) * P for h in range(2)]
    ohbase = np.concatenate([[0], np.cumsum(nch_b)[:-1]])

    X = np.asarray(features)[0]
    Xa = np.asarray(aug_features)[0]
    XT = np.ascontiguousarray(X.T).astype(ml_dtypes.bfloat16)    # [F, N]
    XTa = np.ascontiguousarray(Xa.T).astype(ml_dtypes.bfloat16)
    W1T = np.ascontiguousarray(np.asarray(W1).T).astype(ml_dtypes.bfloat16)
    WtT = np.ascontiguousarray(np.asarray(Wt).T).astype(ml_dtypes.bfloat16)

    def wrap_idx(a):
        # [L] -> [128, L/16]: element i at [i%16, i//16], replicated x8
        L = a.shape[0]
        w = a.reshape(L // 16, 16).T
        return np.ascontiguousarray(np.tile(w, (8, 1)))

    in_maps = []
    for c in range(NC):
        r, cl, g, b, h, key = per_core[c]
        cnt = cnts[c]
        run_start = np.zeros(NB * 2, dtype=np.int64)
        flat = cnt.reshape(-1)
        run_start[1:] = np.cumsum(flat)[:-1]
        rank = np.arange(len(r)) - run_start[key]
        lane = rank % P
        j = rank // P

        idx_streams = []
        for hh in range(2):
            arr = np.zeros(Lh[hh], dtype=np.int16)
            m = h == hh
            off = (strm_base[hh][b[m]] + j[m]) * P + lane[m]
            arr[off] = (cl[m] - hh * HALF).astype(np.int16)
            idx_streams.append(wrap_idx(arr))

        oh = np.zeros((P, NCHT, P), dtype=ml_dtypes.bfloat16)
        ohcol = ohbase[b] + h * CBH[b, 0] + j
        dest = r - b * P
        oh[lane, ohcol, dest] = g.astype(ml_dtypes.bfloat16)

        in_maps.append({
            "xt": np.ascontiguousarray(XT[:, c * NSH:(c + 1) * NSH]),
            "xta": np.ascontiguousarray(XTa[:, c * NSH:(c + 1) * NSH]),
            "w1t": W1T,
            "wtt": WtT,
            "oh": oh,
            "idx0": idx_streams[0],
            "idx1": idx_streams[1],
        })

    meta = dict(
        N=N, F=F, D=D, K=K, NSH=NSH, NB=NB, HALF=HALF, DT=D // P,
        CBH=tuple(map(tuple, CBH.tolist())), NCHT=NCHT,
        strm_base=tuple(map(tuple, strm_base.tolist())),
        Lh=tuple(Lh), ohbase=tuple(ohbase.tolist()),
        FT=cdiv(F, P),
    )
    return in_maps, meta


# --------------------------------------------------------------------------
# device program
# --------------------------------------------------------------------------

def build(meta, debug=False):
    N, F, D, K = meta["N"], meta["F"], meta["D"], meta["K"]
    NSH, NB, HALF, DT = meta["NSH"], meta["NB"], meta["HALF"], meta["DT"]
    CBH = meta["CBH"]
    NCHT = meta["NCHT"]
    strm_base = meta["strm_base"]
    Lh = meta["Lh"]
    ohbase = meta["ohbase"]
    FT = meta["FT"]
    GB = 2                      # blocks per gather group
    W2 = 2 * D                  # combined table row elems (bf16)
    VLEN = 2 * D

    nc = bacc.Bacc("TRN2", target_bir_lowering=False, debug=debug,
                   num_devices=NC)

    xt = nc.dram_tensor("xt", [F, NSH], bf16, kind="ExternalInput")
    xta = nc.dram_tensor("xta", [F, NSH], bf16, kind="ExternalInput")
    w1t = nc.dram_tensor("w1t", [F, D], bf16, kind="ExternalInput")
    wtt = nc.dram_tensor("wtt", [D, K], bf16, kind="ExternalInput")
    oh = nc.dram_tensor("oh", [P, NCHT, P], bf16, kind="ExternalInput")
    idx_d = [nc.dram_tensor(f"idx{h}", [P, Lh[h] // 16], i16, kind="ExternalInput")
             for h in range(2)]

    stats_p_d = nc.dram_tensor("stats_p", [P, 2 * DT], f32, kind="ExternalOutput")
    stats_v_d = nc.dram_tensor("stats_v", [1, VLEN], f32, kind="ExternalOutput")
    s_out_d = nc.dram_tensor("s_out", [NB * P, K], f32, kind="ExternalOutput")

    max_nch_b = max(CBH[b][0] + CBH[b][1] for b in range(NB))
    grp = {h: [] for h in range(2)}   # (start_chunk, nchunks) per group
    for h in range(2):
        for g0 in range(0, NB, GB):
            s = strm_base[h][g0]
            n = sum(CBH[b][h] for b in range(g0, min(g0 + GB, NB)))
            grp[h].append((s, n))
    max_grp = max((n for h in range(2) for (_, n) in grp[h]), default=1)

    with tile.TileContext(nc) as tc:
        with (
            tc.tile_pool(name="big", bufs=4) as bigp,
            tc.tile_pool(name="ohp", bufs=3) as ohp,
            tc.tile_pool(name="persist", bufs=1) as persist,
            tc.tile_pool(name="stage", bufs=3) as stagep,
            tc.tile_pool(name="tmp", bufs=3) as tmpp,
            tc.tile_pool(name="small", bufs=4) as smallp,
            tc.tile_pool(name="svp", bufs=1) as svp,
            tc.tile_pool(name="pa", bufs=3, space="PSUM") as pa,
            tc.tile_pool(name="pb", bufs=4, space="PSUM") as pb,
            tc.tile_pool(name="dram", bufs=1, space="DRAM") as dramp,
        ):
            # ---- constants / resident tensors
            ident = persist.tile([P, P], f32)
            make_identity(nc, ident[:])
            w1t_t = persist.tile([P, FT, D], bf16)
            for t in range(FT):
                fr = min(P, F - t * P)
                nc.sync.dma_start(w1t_t[:fr, t, :], w1t[t * P:t * P + fr, :])
            wtt_t = persist.tile([P, DT, K], bf16)
            for t in range(DT):
                nc.sync.dma_start(wtt_t[:, t, :], wtt[t * P:(t + 1) * P, :])
            idx_t = []
            for h in range(2):
                it = persist.tile([P, Lh[h] // 16], i16, tag=f"idx{h}")
                nc.sync.dma_start(it[:], idx_d[h][:])
                idx_t.append(it)

            ln_la = persist.tile([P, 1], f32, tag="lnla")
            nc.vector.memset(ln_la[:], LN_SELU_LA)
            la_c = persist.tile([P, 1], f32, tag="lac")
            nc.vector.memset(la_c[:], SELU_LA)

            gcn1T = persist.tile([P, DT, NB * P], bf16, tag="gcn1T")
            accs = persist.tile([P, VLEN], f32, tag="accs")
            nc.vector.memset(accs[:], 0.0)
            stats_p = persist.tile([P, 2 * DT], f32, tag="statsp")

            cc_in = dramp.tile([NSH, W2], bf16)
            cc_out = dramp.tile([N, W2], bf16, addr_space="Shared")

            # ================= phase A: h1|h2 = [X|Xa] @ W1^T =============
            for which, src in enumerate((xt, xta)):
                xt_tiles = []
                for t in range(FT):
                    fr = min(P, F - t * P)
                    xx = bigp.tile([P, NSH], bf16, tag="big")
                    nc.sync.dma_start(xx[:fr, :], src[t * P:t * P + fr, :])
                    xt_tiles.append(xx)
                for b in range(NB):
                    rows = min(P, NSH - b * P)
                    pt = pb.tile([P, D], f32, space="PSUM", tag="pb")
                    for t in range(FT):
                        fr = min(P, F - t * P)
                        nc.tensor.matmul(
                            pt[:rows, :],
                            lhsT=xt_tiles[t][:fr, b * P:b * P + rows],
                            rhs=w1t_t[:fr, t, :],
                            start=(t == 0), stop=(t == FT - 1),
                        )
                    st = stagep.tile([P, D], bf16, tag="stage")
                    nc.vector.tensor_copy(st[:rows, :], pt[:rows, :])
                    nc.sync.dma_start(
                        cc_in[b * P:b * P + rows, which * D:(which + 1) * D],
                        st[:rows, :])

            nc.gpsimd.collective_compute(
                "AllGather", mybir.AluOpType.bypass,
                replica_groups=[list(range(NC))],
                ins=[cc_in[:]], outs=[cc_out[:]],
            )

            # ================= phase B: fused SpMM + epilogues =============
            def selu_into(dst_ap, psum_ap):
                """dst = selu(psum), mostly on ScalarE.

                e2 = exp(x + ln(la));  e3 = relu(la - e2);  r = relu(l*x)
                selu = r - e3
                """
                e2 = tmpp.tile([P, D], f32, tag="tmpd")
                nc.scalar.activation(e2[:], psum_ap,
                                     mybir.ActivationFunctionType.Exp,
                                     bias=ln_la[:])
                e3 = tmpp.tile([P, D], f32, tag="tmpd2")
                nc.scalar.activation(e3[:], e2[:],
                                     mybir.ActivationFunctionType.Relu,
                                     bias=la_c[:], scale=-1.0)
                r = tmpp.tile([P, D], f32, tag="tmpd3")
                nc.scalar.activation(r[:], psum_ap,
                                     mybir.ActivationFunctionType.Relu,
                                     scale=SELU_L)
                nc.vector.tensor_tensor(dst_ap, r[:], e3[:],
                                        mybir.AluOpType.subtract)

            gtile = {}

            def get_gtile(h, g):
                if (h, g) not in gtile:
                    s, n = grp[h][g]
                    gt = bigp.tile([P, max_grp, W2], bf16, tag="big")
                    if n > 0:
                        nidx = n * P
                        view = cc_out[h * HALF:(h + 1) * HALF, :]
                        nc.gpsimd.dma_gather(
                            gt[:, 0:n, :], view, idx_t[h][:, s * 8:(s + n) * 8],
                            num_idxs=nidx, num_idxs_reg=nidx, elem_size=W2,
                            single_packet=False,
                        )
                    gtile[(h, g)] = (gt, s)
                return gtile[(h, g)]

            for b in range(NB):
                rows = min(P, NSH - b * P)
                nch = CBH[b][0] + CBH[b][1]
                oht = ohp.tile([P, max_nch_b, P], bf16, tag="oh")
                nc.sync.dma_start(oht[:, 0:nch, :],
                                  oh[:, ohbase[b]:ohbase[b] + nch, :])
                pt = pa.tile([P, W2], f32, space="PSUM", tag="pa")
                nmm = 0
                tot = CBH[b][0] + CBH[b][1]
                if tot == 0:
                    nc.vector.memset(pt[:], 0.0)
                for h in range(2):
                    g = b // GB
                    for j in range(CBH[b][h]):
                        gt, s = get_gtile(h, g)
                        c = strm_base[h][b] + j - s
                        nc.tensor.matmul(
                            pt[:], lhsT=oht[:, h * CBH[b][0] + j, :],
                            rhs=gt[:, c, :],
                            start=(nmm == 0), stop=(nmm == tot - 1))
                        nmm += 1

                # epilogue
                g1b = tmpp.tile([P, D], f32, tag="g1b")
                selu_into(g1b[:], pt[:, 0:D])
                aug = tmpp.tile([P, D], f32, tag="aug")
                selu_into(aug[:], pt[:, D:W2])
                # con partials
                nc.vector.tensor_tensor(accs[:, 0:D], accs[:, 0:D], aug[:],
                                        mybir.AluOpType.add)
                pr = tmpp.tile([P, D], f32, tag="pr")
                nc.vector.tensor_tensor(pr[:], aug[:], g1b[:],
                                        mybir.AluOpType.mult)
                nc.vector.tensor_tensor(accs[:, D:W2], accs[:, D:W2], pr[:],
                                        mybir.AluOpType.add)
                # transposes into gcn1T
                for t in range(DT):
                    ptr = pb.tile([P, P], f32, space="PSUM", tag="pb")
                    nc.tensor.transpose(ptr[:], g1b[:, t * P:(t + 1) * P],
                                        ident[:])
                    nc.vector.tensor_copy(gcn1T[:, t, b * P:(b + 1) * P], ptr[:])
                # logits -> softmax -> S -> DRAM
                pl = pb.tile([P, K], f32, space="PSUM", tag="pb")
                for t in range(DT):
                    nc.tensor.matmul(pl[:], lhsT=gcn1T[:, t, b * P:(b + 1) * P],
                                     rhs=wtt_t[:, t, :],
                                     start=(t == 0), stop=(t == DT - 1))
                nmx = smallp.tile([P, 1], f32, tag="nmx")
                nc.vector.reduce_max(nmx[:], pl[:], axis=mybir.AxisListType.X,
                                     negate=True)
                ex = smallp.tile([P, K], f32, tag="ex")
                sm = smallp.tile([P, 1], f32, tag="sm")
                nc.scalar.activation(ex[:], pl[:],
                                     mybir.ActivationFunctionType.Exp,
                                     bias=nmx[:], accum_out=sm[:])
                rc = smallp.tile([P, 1], f32, tag="rc")
                nc.vector.reciprocal(rc[:], sm[:])
                sb = stagep.tile([P, K], f32, tag="sstage")
                nc.scalar.mul(sb[:], ex[:], rc[:])
                nc.sync.dma_start(s_out_d[b * P:(b + 1) * P, :], sb[:])

            # ================= phase E: stats =================
            for t in range(DT):
                nmt = smallp.tile([P, 1], f32, tag="nmt")
                nc.vector.reduce_max(nmt[:], gcn1T[:, t, 0:NSH],
                                     axis=mybir.AxisListType.X, negate=True)
                nc.scalar.mul(stats_p[:, t:t + 1], nmt[:], -1.0)
                exb = bigp.tile([P, NSH], bf16, tag="big")
                nc.scalar.activation(exb[:, 0:NSH], gcn1T[:, t, 0:NSH],
                                     mybir.ActivationFunctionType.Exp,
                                     bias=nmt[:],
                                     accum_out=stats_p[:, DT + t:DT + t + 1])
            sv = svp.tile([1, VLEN], f32, tag="sv")
            nc.gpsimd.tensor_reduce(sv[:], accs[:], axis=mybir.AxisListType.C,
                                    op=mybir.AluOpType.add)
            nc.sync.dma_start(stats_v_d[:], sv[:])
            nc.sync.dma_start(stats_p_d[:], stats_p[:])

    nc.compile()
    return nc


# --------------------------------------------------------------------------
# host-side combine of per-core partials
# --------------------------------------------------------------------------

def combine(results, cfg, graph_row, graph_col, gn_vals):
    N, D, K = cfg["N"], cfg["D"], cfg["K"]
    NSH = N // NC
    DT = D // P
    E = float(graph_row.shape[0])

    m, s = [], []
    colsum_aug = np.zeros(D)
    dot = 0.0
    S_full = np.zeros((N, K))
    for c in range(NC):
        sp = np.asarray(results[c]["stats_p"], dtype=np.float64)
        svv = np.asarray(results[c]["stats_v"], dtype=np.float64).reshape(-1)
        m.append(np.concatenate([sp[:, t] for t in range(DT)]))
        s.append(np.concatenate([sp[:, DT + t] for t in range(DT)]))
        colsum_aug += svv[0:D]
        dot += svv[D:2 * D].sum()
        S_full[c * NSH:(c + 1) * NSH] = \
            np.asarray(results[c]["s_out"], dtype=np.float64)[:NSH]
    m = np.stack(m)
    s = np.stack(s)
    M = m.max(axis=0)
    Sg = (np.exp(m - M) * s).sum(axis=0)
    logZ = M + np.log(Sg)

    row = np.asarray(graph_row).astype(np.int64)
    col = np.asarray(graph_col).astype(np.int64)
    deg = np.bincount(col, minlength=N).astype(np.float64)

    trace_gp = np.einsum('ek,ek->', S_full[row], S_full[col])
    nl = S_full.T @ deg
    clsz = S_full.sum(axis=0)

    spectral = -(trace_gp - (nl ** 2).sum() / (2.0 * E)) / (2.0 * E)
    cluster = (np.linalg.norm(clsz) / N * math.sqrt(K) - 1.0) * CLUSTER_REG
    con = -(dot - (logZ * colsum_aug).sum()) / D
    return spectral + cluster + CON_REG * con


# --------------------------------------------------------------------------
# entry point
# --------------------------------------------------------------------------

_BUILD_CACHE = {}


def kernel(features, aug_features, graph_row, graph_col, graph_vals, gn_vals,
           lbl, dense_graph, W1, b1, Wt, bt, _cfg=None, _trace=False):
    cfg = _cfg or FULL
    in_maps, meta = prep(features, aug_features, graph_row, graph_col,
                         gn_vals, W1, Wt, cfg)
    key = tuple(sorted((k, str(v)) for k, v in meta.items()))
    if key not in _BUILD_CACHE:
        _BUILD_CACHE[key] = build(meta)
    nc = _BUILD_CACHE[key]
    res = bass_utils.run_bass_kernel_spmd(nc, in_maps, core_ids=list(range(NC)),
                                          trace=_trace)
    loss = combine(res.results, cfg, graph_row, graph_col, gn_vals)
    out = np.array(loss, dtype=np.float32)
    if _trace:
        return out, res
    return out


# revision 11
# speedup vs baseline: 2.3707x; 1.1128x over previous
"""Trainium2 8-core kernel for nn_CAT_81269371175150 (GNN message passing).

Math (see reference):
  gcn(x)   = selu(A_gn @ (x @ W1^T))            for features and aug_features
  S        = softmax_K(gcn1 @ Wt^T)
  loss     = spectral(S, A) + cluster(S) + 0.5 * con(gcn1, gcn2)

Strategy (v2):
  * Nodes sharded row-wise across 8 cores; edge list bucketed by destination
    shard, sorted by (dest block, src half, src col), padded to fixed
    chunk counts so all cores run one SPMD program.
  * h1 = X @ W1^T and h2 = Xa @ W1^T computed per-shard (bf16), packed as
    combined 1KB table rows [h1 | h2], AllGather'd to every core's HBM.
  * Single SpMM pass: dma_gather of table[col] rows (1KB) + one matmul per
    128-edge chunk (one-hot lhsT with gn values folded in) accumulating
    [A@h1 | A@h2] into one PSUM bank per destination block.
  * selu runs mostly on ScalarE; S = softmax_K is computed per block and
    written out; log-softmax stats + con-loss partials accumulate on-chip.
  * Host finishes the tiny reductions: trace(S^T A S), nl, cluster sizes,
    log-softmax renormalization across cores, final scalar.
"""

import math
import numpy as np
import ml_dtypes

import concourse.bacc as bacc
import concourse.mybir as mybir
import concourse.tile as tile
from concourse import bass_utils
from concourse.masks import make_identity

P = 128
NC = 8

# full-size problem constants
FULL = dict(N=50000, F=500, D=256, K=16)

SELU_L = 1.0507009873554805
SELU_A = 1.6732632423543772
SELU_LA = SELU_L * SELU_A
LN_SELU_LA = math.log(SELU_LA)

CLUSTER_REG = 1.0
CON_REG = 0.5

bf16 = mybir.dt.bfloat16
f32 = mybir.dt.float32
i16 = mybir.dt.int16


def cdiv(a, b):
    return -(-a // b)


# --------------------------------------------------------------------------
# host-side preprocessing
# --------------------------------------------------------------------------

def prep(features, aug_features, graph_row, graph_col, gn_vals, W1, Wt, cfg):
    N, F, D, K = cfg["N"], cfg["F"], cfg["D"], cfg["K"]
    NSH = N // NC
    NB = cdiv(NSH, P)
    HALF = N // 2

    row = np.asarray(graph_row).astype(np.int64)
    col = np.asarray(graph_col).astype(np.int64)
    gn = np.asarray(gn_vals).astype(np.float64)

    core = row // NSH
    per_core = []
    cnts = np.zeros((NC, NB, 2), dtype=np.int64)
    for c in range(NC):
        m = core == c
        r = row[m] - c * NSH
        cl = col[m]
        g = gn[m]
        b = r // P
        h = cl // HALF
        order = np.lexsort((cl, h, b))
        r, cl, g, b, h = r[order], cl[order], g[order], b[order], h[order]
        key = b * 2 + h
        cnt = np.bincount(key, minlength=NB * 2).reshape(NB, 2)
        cnts[c] = cnt
        per_core.append((r, cl, g, b, h, key))

    CBH = np.ceil(cnts.max(axis=0) / P).astype(np.int64)        # [NB, 2]
    nch_b = CBH[:, 0] + CBH[:, 1]
    NCHT = int(nch_b.sum())
    strm_base = np.zeros((2, NB), dtype=np.int64)
    for h in range(2):
        strm_base[h] = np.concatenate([[0], np.cumsum(CBH[:, h])[:-1]])
    Lh = [int(CBH[:, h].sum()) * P for h in range(2)]
    ohbase = np.concatenate([[0], np.cumsum(nch_b)[:-1]])

    X = np.asarray(features)[0]
    Xa = np.asarray(aug_features)[0]
    XT = np.ascontiguousarray(X.T).astype(ml_dtypes.bfloat16)    # [F, N]
    XTa = np.ascontiguousarray(Xa.T).astype(ml_dtypes.bfloat16)
    W1T = np.ascontiguousarray(np.asarray(W1).T).astype(ml_dtypes.bfloat16)
    WtT = np.ascontiguousarray(np.asarray(Wt).T).astype(ml_dtypes.bfloat16)

    def wrap_idx(a):
        # [L] -> [128, L/16]: element i at [i%16, i//16], replicated x8
        L = a.shape[0]
        w = a.reshape(L // 16, 16).T
        return np.ascontiguousarray(np.tile(w, (8, 1)))

    in_maps = []
    for c in range(NC):
        r, cl, g, b, h, key = per_core[c]
        cnt = cnts[c]
        run_start = np.zeros(NB * 2, dtype=np.int64)
        flat = cnt.reshape(-1)
        run_start[1:] = np.cumsum(flat)[:-1]
        rank = np.arange(len(r)) - run_start[key]
        lane = rank % P
        j = rank // P

        idx_streams = []
        for hh in range(2):
            arr = np.zeros(Lh[hh], dtype=np.int16)
            m = h == hh
            off = (strm_base[hh][b[m]] + j[m]) * P + lane[m]
            arr[off] = (cl[m] - hh * HALF).astype(np.int16)
            idx_streams.append(wrap_idx(arr))

        oh = np.zeros((P, NCHT, P), dtype=ml_dtypes.bfloat16)
        ohcol = ohbase[b] + h * CBH[b, 0] + j
        dest = r - b * P
        oh[lane, ohcol, dest] = g.astype(ml_dtypes.bfloat16)

        in_maps.append({
            "xt": np.ascontiguousarray(XT[:, c * NSH:(c + 1) * NSH]),
            "xta": np.ascontiguousarray(XTa[:, c * NSH:(c + 1) * NSH]),
            "w1t": W1T,
            "wtt": WtT,
            "oh": oh,
            "idx0": idx_streams[0],
            "idx1": idx_streams[1],
        })

    meta = dict(
        N=N, F=F, D=D, K=K, NSH=NSH, NB=NB, HALF=HALF, DT=D // P,
        CBH=tuple(map(tuple, CBH.tolist())), NCHT=NCHT,
        strm_base=tuple(map(tuple, strm_base.tolist())),
        Lh=tuple(Lh), ohbase=tuple(ohbase.tolist()),
        FT=cdiv(F, P),
    )
    return in_maps, meta


# --------------------------------------------------------------------------
# device program
# --------------------------------------------------------------------------

def build(meta, debug=False):
    N, F, D, K = meta["N"], meta["F"], meta["D"], meta["K"]
    NSH, NB, HALF, DT = meta["NSH"], meta["NB"], meta["HALF"], meta["DT"]
    CBH = meta["CBH"]
    NCHT = meta["NCHT"]
    strm_base = meta["strm_base"]
    Lh = meta["Lh"]
    ohbase = meta["ohbase"]
    FT = meta["FT"]
    GB = 2                      # blocks per gather group
    W2 = 2 * D                  # combined table row elems (bf16)
    VLEN = 2 * D

    nc = bacc.Bacc("TRN2", target_bir_lowering=False, debug=debug,
                   num_devices=NC, num_swdge_queues=4)

    xt = nc.dram_tensor("xt", [F, NSH], bf16, kind="ExternalInput")
    xta = nc.dram_tensor("xta", [F, NSH], bf16, kind="ExternalInput")
    w1t = nc.dram_tensor("w1t", [F, D], bf16, kind="ExternalInput")
    wtt = nc.dram_tensor("wtt", [D, K], bf16, kind="ExternalInput")
    oh = nc.dram_tensor("oh", [P, NCHT, P], bf16, kind="ExternalInput")
    idx_d = [nc.dram_tensor(f"idx{h}", [P, Lh[h] // 16], i16, kind="ExternalInput")
             for h in range(2)]

    stats_p_d = nc.dram_tensor("stats_p", [P, 2 * DT], f32, kind="ExternalOutput")
    stats_v_d = nc.dram_tensor("stats_v", [1, VLEN], f32, kind="ExternalOutput")
    s_out_d = nc.dram_tensor("s_out", [NB * P, K], f32, kind="ExternalOutput")

    max_nch_b = max(CBH[b][0] + CBH[b][1] for b in range(NB))
    grp = {h: [] for h in range(2)}   # (start_chunk, nchunks) per group
    for h in range(2):
        for g0 in range(0, NB, GB):
            s = strm_base[h][g0]
            n = sum(CBH[b][h] for b in range(g0, min(g0 + GB, NB)))
            grp[h].append((s, n))
    max_grp = max((n for h in range(2) for (_, n) in grp[h]), default=1)

    with tile.TileContext(nc) as tc:
        with (
            tc.tile_pool(name="big", bufs=4) as bigp,
            tc.tile_pool(name="ohp", bufs=3) as ohp,
            tc.tile_pool(name="persist", bufs=1) as persist,
            tc.tile_pool(name="stage", bufs=3) as stagep,
            tc.tile_pool(name="tmp", bufs=3) as tmpp,
            tc.tile_pool(name="small", bufs=4) as smallp,
            tc.tile_pool(name="svp", bufs=1) as svp,
            tc.tile_pool(name="pa", bufs=3, space="PSUM") as pa,
            tc.tile_pool(name="pb", bufs=4, space="PSUM") as pb,
            tc.tile_pool(name="dram", bufs=1, space="DRAM") as dramp,
        ):
            # ---- constants / resident tensors
            ident = persist.tile([P, P], f32)
            make_identity(nc, ident[:])
            w1t_t = persist.tile([P, FT, D], bf16)
            for t in range(FT):
                fr = min(P, F - t * P)
                nc.sync.dma_start(w1t_t[:fr, t, :], w1t[t * P:t * P + fr, :])
            wtt_t = persist.tile([P, DT, K], bf16)
            for t in range(DT):
                nc.sync.dma_start(wtt_t[:, t, :], wtt[t * P:(t + 1) * P, :])
            idx_t = []
            for h in range(2):
                it = persist.tile([P, Lh[h] // 16], i16, tag=f"idx{h}")
                nc.sync.dma_start(it[:], idx_d[h][:])
                idx_t.append(it)

            ln_la = persist.tile([P, 1], f32, tag="lnla")
            nc.vector.memset(ln_la[:], LN_SELU_LA)
            la_c = persist.tile([P, 1], f32, tag="lac")
            nc.vector.memset(la_c[:], SELU_LA)

            gcn1T = persist.tile([P, DT, NB * P], bf16, tag="gcn1T")
            accs = persist.tile([P, VLEN], f32, tag="accs")
            nc.vector.memset(accs[:], 0.0)
            stats_p = persist.tile([P, 2 * DT], f32, tag="statsp")

            cc_in = dramp.tile([NSH, W2], bf16)
            cc_out = dramp.tile([N, W2], bf16, addr_space="Shared")

            # ================= phase A: h1|h2 = [X|Xa] @ W1^T =============
            for which, src in enumerate((xt, xta)):
                xt_tiles = []
                for t in range(FT):
                    fr = min(P, F - t * P)
                    xx = bigp.tile([P, NSH], bf16, tag="big")
                    nc.sync.dma_start(xx[:fr, :], src[t * P:t * P + fr, :])
                    xt_tiles.append(xx)
                for b in range(NB):
                    rows = min(P, NSH - b * P)
                    pt = pb.tile([P, D], f32, space="PSUM", tag="pb")
                    for t in range(FT):
                        fr = min(P, F - t * P)
                        nc.tensor.matmul(
                            pt[:rows, :],
                            lhsT=xt_tiles[t][:fr, b * P:b * P + rows],
                            rhs=w1t_t[:fr, t, :],
                            start=(t == 0), stop=(t == FT - 1),
                        )
                    st = stagep.tile([P, D], bf16, tag="stage")
                    nc.vector.tensor_copy(st[:rows, :], pt[:rows, :])
                    nc.sync.dma_start(
                        cc_in[b * P:b * P + rows, which * D:(which + 1) * D],
                        st[:rows, :])

            nc.gpsimd.collective_compute(
                "AllGather", mybir.AluOpType.bypass,
                replica_groups=[list(range(NC))],
                ins=[cc_in[:]], outs=[cc_out[:]],
            )

            # ================= phase B: fused SpMM + epilogues =============
            def selu_into(dst_ap, psum_ap):
                """dst = selu(psum), mostly on ScalarE.

                e2 = exp(x + ln(la));  e3 = relu(la - e2);  r = relu(l*x)
                selu = r - e3
                """
                e2 = tmpp.tile([P, D], f32, tag="tmpd")
                nc.scalar.activation(e2[:], psum_ap,
                                     mybir.ActivationFunctionType.Exp,
                                     bias=ln_la[:])
                e3 = tmpp.tile([P, D], f32, tag="tmpd2")
                nc.scalar.activation(e3[:], e2[:],
                                     mybir.ActivationFunctionType.Relu,
                                     bias=la_c[:], scale=-1.0)
                r = tmpp.tile([P, D], f32, tag="tmpd3")
                nc.scalar.activation(r[:], psum_ap,
                                     mybir.ActivationFunctionType.Relu,
                                     scale=SELU_L)
                nc.vector.tensor_tensor(dst_ap, r[:], e3[:],
                                        mybir.AluOpType.subtract)

            gtile = {}

            def get_gtile(h, g):
                if (h, g) not in gtile:
                    s, n = grp[h][g]
                    gt = bigp.tile([P, max_grp, W2], bf16, tag="big")
                    if n > 0:
                        nidx = n * P
                        view = cc_out[h * HALF:(h + 1) * HALF, :]
                        nc.gpsimd.dma_gather(
                            gt[:, 0:n, :], view, idx_t[h][:, s * 8:(s + n) * 8],
                            num_idxs=nidx, num_idxs_reg=nidx, elem_size=W2,
                            single_packet=False,
                            queue_num=(h * 2 + g % 2),
                        )
                    gtile[(h, g)] = (gt, s)
                return gtile[(h, g)]

            for b in range(NB):
                rows = min(P, NSH - b * P)
                nch = CBH[b][0] + CBH[b][1]
                oht = ohp.tile([P, max_nch_b, P], bf16, tag="oh")
                nc.sync.dma_start(oht[:, 0:nch, :],
                                  oh[:, ohbase[b]:ohbase[b] + nch, :])
                pt = pa.tile([P, W2], f32, space="PSUM", tag="pa")
                nmm = 0
                tot = CBH[b][0] + CBH[b][1]
                if tot == 0:
                    nc.vector.memset(pt[:], 0.0)
                for h in range(2):
                    g = b // GB
                    for j in range(CBH[b][h]):
                        gt, s = get_gtile(h, g)
                        c = strm_base[h][b] + j - s
                        nc.tensor.matmul(
                            pt[:], lhsT=oht[:, h * CBH[b][0] + j, :],
                            rhs=gt[:, c, :],
                            start=(nmm == 0), stop=(nmm == tot - 1))
                        nmm += 1

                # epilogue
                g1b = tmpp.tile([P, D], f32, tag="g1b")
                selu_into(g1b[:], pt[:, 0:D])
                aug = tmpp.tile([P, D], f32, tag="aug")
                selu_into(aug[:], pt[:, D:W2])
                # con partials
                nc.vector.tensor_tensor(accs[:, 0:D], accs[:, 0:D], aug[:],
                                        mybir.AluOpType.add)
                pr = tmpp.tile([P, D], f32, tag="pr")
                nc.vector.tensor_tensor(pr[:], aug[:], g1b[:],
                                        mybir.AluOpType.mult)
                nc.vector.tensor_tensor(accs[:, D:W2], accs[:, D:W2], pr[:],
                                        mybir.AluOpType.add)
                # transposes into gcn1T
                for t in range(DT):
                    ptr = pb.tile([P, P], f32, space="PSUM", tag="pb")
                    nc.tensor.transpose(ptr[:], g1b[:, t * P:(t + 1) * P],
                                        ident[:])
                    nc.vector.tensor_copy(gcn1T[:, t, b * P:(b + 1) * P], ptr[:])
                # logits -> softmax -> S -> DRAM
                pl = pb.tile([P, K], f32, space="PSUM", tag="pb")
                for t in range(DT):
                    nc.tensor.matmul(pl[:], lhsT=gcn1T[:, t, b * P:(b + 1) * P],
                                     rhs=wtt_t[:, t, :],
                                     start=(t == 0), stop=(t == DT - 1))
                nmx = smallp.tile([P, 1], f32, tag="nmx")
                nc.vector.reduce_max(nmx[:], pl[:], axis=mybir.AxisListType.X,
                                     negate=True)
                ex = smallp.tile([P, K], f32, tag="ex")
                sm = smallp.tile([P, 1], f32, tag="sm")
                nc.scalar.activation(ex[:], pl[:],
                                     mybir.ActivationFunctionType.Exp,
                                     bias=nmx[:], accum_out=sm[:])
                rc = smallp.tile([P, 1], f32, tag="rc")
                nc.vector.reciprocal(rc[:], sm[:])
                sb = stagep.tile([P, K], f32, tag="sstage")
                nc.scalar.mul(sb[:], ex[:], rc[:])
                nc.sync.dma_start(s_out_d[b * P:(b + 1) * P, :], sb[:])

            # ================= phase E: stats =================
            for t in range(DT):
                nmt = smallp.tile([P, 1], f32, tag="nmt")
                nc.vector.reduce_max(nmt[:], gcn1T[:, t, 0:NSH],
                                     axis=mybir.AxisListType.X, negate=True)
                nc.scalar.mul(stats_p[:, t:t + 1], nmt[:], -1.0)
                exb = bigp.tile([P, NSH], bf16, tag="big")
                nc.scalar.activation(exb[:, 0:NSH], gcn1T[:, t, 0:NSH],
                                     mybir.ActivationFunctionType.Exp,
                                     bias=nmt[:],
                                     accum_out=stats_p[:, DT + t:DT + t + 1])
            ones = persist.tile([P, 1], f32, tag="ones")
            nc.vector.memset(ones[:], 1.0)
            pv = pa.tile([P, VLEN], f32, space="PSUM", tag="pa")
            nc.tensor.matmul(pv[0:1, :], lhsT=ones[:], rhs=accs[:],
                             start=True, stop=True)
            sv = svp.tile([1, VLEN], f32, tag="sv")
            nc.vector.tensor_copy(sv[:], pv[0:1, :])
            nc.sync.dma_start(stats_v_d[:], sv[:])
            nc.sync.dma_start(stats_p_d[:], stats_p[:])

    nc.compile()
    return nc


# --------------------------------------------------------------------------
# host-side combine of per-core partials
# --------------------------------------------------------------------------

def combine(results, cfg, graph_row, graph_col, gn_vals):
    N, D, K = cfg["N"], cfg["D"], cfg["K"]
    NSH = N // NC
    DT = D // P
    E = float(graph_row.shape[0])

    m, s = [], []
    colsum_aug = np.zeros(D)
    dot = 0.0
    S_full = np.zeros((N, K))
    for c in range(NC):
        sp = np.asarray(results[c]["stats_p"], dtype=np.float64)
        svv = np.asarray(results[c]["stats_v"], dtype=np.float64).reshape(-1)
        m.append(np.concatenate([sp[:, t] for t in range(DT)]))
        s.append(np.concatenate([sp[:, DT + t] for t in range(DT)]))
        colsum_aug += svv[0:D]
        dot += svv[D:2 * D].sum()
        S_full[c * NSH:(c + 1) * NSH] = \
            np.asarray(results[c]["s_out"], dtype=np.float64)[:NSH]
    m = np.stack(m)
    s = np.stack(s)
    M = m.max(axis=0)
    Sg = (np.exp(m - M) * s).sum(axis=0)
    logZ = M + np.log(Sg)

    row = np.asarray(graph_row).astype(np.int64)
    col = np.asarray(graph_col).astype(np.int64)
    deg = np.bincount(col, minlength=N).astype(np.float64)

    trace_gp = np.einsum('ek,ek->', S_full[row], S_full[col])
    nl = S_full.T @ deg
    clsz = S_full.sum(axis=0)

    spectral = -(trace_gp - (nl ** 2).sum() / (2.0 * E)) / (2.0 * E)
    cluster = (np.linalg.norm(clsz) / N * math.sqrt(K) - 1.0) * CLUSTER_REG
    con = -(dot - (logZ * colsum_aug).sum()) / D
    return spectral + cluster + CON_REG * con


# --------------------------------------------------------------------------
# entry point
# --------------------------------------------------------------------------

_BUILD_CACHE = {}


def kernel(features, aug_features, graph_row, graph_col, graph_vals, gn_vals,
           lbl, dense_graph, W1, b1, Wt, bt, _cfg=None, _trace=False):
    cfg = _cfg or FULL
    in_maps, meta = prep(features, aug_features, graph_row, graph_col,
                         gn_vals, W1, Wt, cfg)
    key = tuple(sorted((k, str(v)) for k, v in meta.items()))
    if key not in _BUILD_CACHE:
        _BUILD_CACHE[key] = build(meta)
    nc = _BUILD_CACHE[key]
    res = bass_utils.run_bass_kernel_spmd(nc, in_maps, core_ids=list(range(NC)),
                                          trace=_trace)
    loss = combine(res.results, cfg, graph_row, graph_col, gn_vals)
    out = np.array(loss, dtype=np.float32)
    if _trace:
        return out, res
    return out


# revision 14
# speedup vs baseline: 2.5172x; 1.0618x over previous
"""Trainium2 8-core kernel for nn_CAT_81269371175150 (GNN message passing).

Math (see reference):
  gcn(x)   = selu(A_gn @ (x @ W1^T))            for features and aug_features
  S        = softmax_K(gcn1 @ Wt^T)
  loss     = spectral(S, A) + cluster(S) + 0.5 * con(gcn1, gcn2)

Strategy (v2):
  * Nodes sharded row-wise across 8 cores; edge list bucketed by destination
    shard, sorted by (dest block, src half, src col), padded to fixed
    chunk counts so all cores run one SPMD program.
  * h1 = X @ W1^T and h2 = Xa @ W1^T computed per-shard (bf16), packed as
    combined 1KB table rows [h1 | h2], AllGather'd to every core's HBM.
  * Single SpMM pass: dma_gather of table[col] rows (1KB) + one matmul per
    128-edge chunk (one-hot lhsT with gn values folded in) accumulating
    [A@h1 | A@h2] into one PSUM bank per destination block.
  * selu runs mostly on ScalarE; S = softmax_K is computed per block and
    written out; log-softmax stats + con-loss partials accumulate on-chip.
  * Host finishes the tiny reductions: trace(S^T A S), nl, cluster sizes,
    log-softmax renormalization across cores, final scalar.
"""

import math
import numpy as np
import ml_dtypes

import concourse.bacc as bacc
import concourse.mybir as mybir
import concourse.tile as tile
from concourse import bass_utils
from concourse.masks import make_identity

P = 128
NC = 8

# full-size problem constants
FULL = dict(N=50000, F=500, D=256, K=16)

SELU_L = 1.0507009873554805
SELU_A = 1.6732632423543772
SELU_LA = SELU_L * SELU_A
LN_SELU_LA = math.log(SELU_LA)

CLUSTER_REG = 1.0
CON_REG = 0.5

bf16 = mybir.dt.bfloat16
f32 = mybir.dt.float32
i16 = mybir.dt.int16


def cdiv(a, b):
    return -(-a // b)


# --------------------------------------------------------------------------
# host-side preprocessing
# --------------------------------------------------------------------------

def prep(features, aug_features, graph_row, graph_col, gn_vals, W1, Wt, cfg):
    N, F, D, K = cfg["N"], cfg["F"], cfg["D"], cfg["K"]
    NSH = N // NC
    NB = cdiv(NSH, P)
    HALF = N // 2

    row = np.asarray(graph_row).astype(np.int64)
    col = np.asarray(graph_col).astype(np.int64)
    gn = np.asarray(gn_vals).astype(np.float64)

    SEG = NSH // 2
    core = row // NSH
    per_core = []
    cnts = np.zeros((NC, NB, 2), dtype=np.int64)
    for c in range(NC):
        m = core == c
        r = row[m] - c * NSH
        cl = col[m]
        g = gn[m]
        b = r // P
        h = (cl % NSH) // SEG
        order = np.lexsort((cl, h, b))
        r, cl, g, b, h = r[order], cl[order], g[order], b[order], h[order]
        key = b * 2 + h
        cnt = np.bincount(key, minlength=NB * 2).reshape(NB, 2)
        cnts[c] = cnt
        per_core.append((r, cl, g, b, h, key))

    CBH = np.ceil(cnts.max(axis=0) / P).astype(np.int64)        # [NB, 2]
    nch_b = CBH[:, 0] + CBH[:, 1]
    NCHT = int(nch_b.sum())
    strm_base = np.zeros((2, NB), dtype=np.int64)
    for h in range(2):
        strm_base[h] = np.concatenate([[0], np.cumsum(CBH[:, h])[:-1]])
    Lh = [int(CBH[:, h].sum()) * P for h in range(2)]
    ohbase = np.concatenate([[0], np.cumsum(nch_b)[:-1]])

    X = np.asarray(features)[0]
    Xa = np.asarray(aug_features)[0]
    XT = np.ascontiguousarray(X.T).astype(ml_dtypes.bfloat16)    # [F, N]
    XTa = np.ascontiguousarray(Xa.T).astype(ml_dtypes.bfloat16)
    W1T = np.ascontiguousarray(np.asarray(W1).T).astype(ml_dtypes.bfloat16)
    WtT = np.ascontiguousarray(np.asarray(Wt).T).astype(ml_dtypes.bfloat16)

    def wrap_idx(a):
        # [L] -> [128, L/16]: element i at [i%16, i//16], replicated x8
        L = a.shape[0]
        w = a.reshape(L // 16, 16).T
        return np.ascontiguousarray(np.tile(w, (8, 1)))

    in_maps = []
    for c in range(NC):
        r, cl, g, b, h, key = per_core[c]
        cnt = cnts[c]
        run_start = np.zeros(NB * 2, dtype=np.int64)
        flat = cnt.reshape(-1)
        run_start[1:] = np.cumsum(flat)[:-1]
        rank = np.arange(len(r)) - run_start[key]
        lane = rank % P
        j = rank // P

        idx_streams = []
        loc = (cl // NSH) * SEG + (cl % NSH) - h * SEG
        for hh in range(2):
            arr = np.zeros(Lh[hh], dtype=np.int16)
            m = h == hh
            off = (strm_base[hh][b[m]] + j[m]) * P + lane[m]
            arr[off] = loc[m].astype(np.int16)
            idx_streams.append(wrap_idx(arr))

        oh = np.zeros((P, NCHT, P), dtype=ml_dtypes.bfloat16)
        ohcol = ohbase[b] + h * CBH[b, 0] + j
        dest = r - b * P
        oh[lane, ohcol, dest] = g.astype(ml_dtypes.bfloat16)

        in_maps.append({
            "xt": np.ascontiguousarray(XT[:, c * NSH:(c + 1) * NSH]),
            "xta": np.ascontiguousarray(XTa[:, c * NSH:(c + 1) * NSH]),
            "w1t": W1T,
            "wtt": WtT,
            "oh": oh,
            "idx0": idx_streams[0],
            "idx1": idx_streams[1],
        })

    meta = dict(
        N=N, F=F, D=D, K=K, NSH=NSH, NB=NB, SEG=SEG, DT=D // P,
        CBH=tuple(map(tuple, CBH.tolist())), NCHT=NCHT,
        strm_base=tuple(map(tuple, strm_base.tolist())),
        Lh=tuple(Lh), ohbase=tuple(ohbase.tolist()),
        FT=cdiv(F, P),
    )
    return in_maps, meta


# --------------------------------------------------------------------------
# device program
# --------------------------------------------------------------------------

def build(meta, debug=False):
    N, F, D, K = meta["N"], meta["F"], meta["D"], meta["K"]
    NSH, NB, SEG, DT = meta["NSH"], meta["NB"], meta["SEG"], meta["DT"]
    CBH = meta["CBH"]
    NCHT = meta["NCHT"]
    strm_base = meta["strm_base"]
    Lh = meta["Lh"]
    ohbase = meta["ohbase"]
    FT = meta["FT"]
    W2 = 2 * D                  # combined table row elems (bf16)
    VLEN = 2 * D

    nc = bacc.Bacc("TRN2", target_bir_lowering=False, debug=debug,
                   num_devices=NC, num_swdge_queues=4)

    xt = nc.dram_tensor("xt", [F, NSH], bf16, kind="ExternalInput")
    xta = nc.dram_tensor("xta", [F, NSH], bf16, kind="ExternalInput")
    w1t = nc.dram_tensor("w1t", [F, D], bf16, kind="ExternalInput")
    wtt = nc.dram_tensor("wtt", [D, K], bf16, kind="ExternalInput")
    oh = nc.dram_tensor("oh", [P, NCHT, P], bf16, kind="ExternalInput")
    idx_d = [nc.dram_tensor(f"idx{h}", [P, Lh[h] // 16], i16, kind="ExternalInput")
             for h in range(2)]

    stats_p_d = nc.dram_tensor("stats_p", [P, 2 * DT], f32, kind="ExternalOutput")
    stats_v_d = nc.dram_tensor("stats_v", [1, VLEN], f32, kind="ExternalOutput")
    s_out_d = nc.dram_tensor("s_out", [NB * P, K], f32, kind="ExternalOutput")

    max_nch_b = max(CBH[b][0] + CBH[b][1] for b in range(NB))
    max_grp = max((CBH[b][h] for b in range(NB) for h in range(2)), default=1)

    with tile.TileContext(nc) as tc:
        with (
            tc.tile_pool(name="big", bufs=6) as bigp,
            tc.tile_pool(name="ohp", bufs=3) as ohp,
            tc.tile_pool(name="persist", bufs=1) as persist,
            tc.tile_pool(name="stage", bufs=3) as stagep,
            tc.tile_pool(name="tmp", bufs=3) as tmpp,
            tc.tile_pool(name="small", bufs=4) as smallp,
            tc.tile_pool(name="svp", bufs=1) as svp,
            tc.tile_pool(name="pa", bufs=3, space="PSUM") as pa,
            tc.tile_pool(name="pb", bufs=4, space="PSUM") as pb,
            tc.tile_pool(name="dram", bufs=1, space="DRAM") as dramp,
        ):
            # ---- constants / resident tensors
            ident = persist.tile([P, P], f32)
            make_identity(nc, ident[:])
            w1t_t = persist.tile([P, FT, D], bf16)
            for t in range(FT):
                fr = min(P, F - t * P)
                nc.sync.dma_start(w1t_t[:fr, t, :], w1t[t * P:t * P + fr, :])
            wtt_t = persist.tile([P, DT, K], bf16)
            for t in range(DT):
                nc.sync.dma_start(wtt_t[:, t, :], wtt[t * P:(t + 1) * P, :])
            idx_t = []
            for h in range(2):
                it = persist.tile([P, Lh[h] // 16], i16, tag=f"idx{h}")
                nc.sync.dma_start(it[:], idx_d[h][:])
                idx_t.append(it)

            ln_la = persist.tile([P, 1], f32, tag="lnla")
            nc.vector.memset(ln_la[:], LN_SELU_LA)
            la_c = persist.tile([P, 1], f32, tag="lac")
            nc.vector.memset(la_c[:], SELU_LA)

            gcn1T = persist.tile([P, DT, NB * P], bf16, tag="gcn1T")
            accs = persist.tile([P, VLEN], f32, tag="accs")
            nc.vector.memset(accs[:], 0.0)
            stats_p = persist.tile([P, 2 * DT], f32, tag="statsp")

            cc_in = [dramp.tile([SEG, W2], bf16, name=f"cc_in{q}")
                     for q in range(2)]
            cc_out = [dramp.tile([NC * SEG, W2], bf16, addr_space="Shared",
                                 name=f"cc_out{q}")
                      for q in range(2)]

            # ================= phase A: h1|h2 = [X|Xa] @ W1^T =============
            for which, src in enumerate((xt, xta)):
                xt_tiles = []
                for t in range(FT):
                    fr = min(P, F - t * P)
                    xx = bigp.tile([P, NSH], bf16, tag="big")
                    nc.sync.dma_start(xx[:fr, :], src[t * P:t * P + fr, :])
                    xt_tiles.append(xx)
                for b in range(NB):
                    rows = min(P, NSH - b * P)
                    pt = pb.tile([P, D], f32, space="PSUM", tag="pb")
                    for t in range(FT):
                        fr = min(P, F - t * P)
                        nc.tensor.matmul(
                            pt[:rows, :],
                            lhsT=xt_tiles[t][:fr, b * P:b * P + rows],
                            rhs=w1t_t[:fr, t, :],
                            start=(t == 0), stop=(t == FT - 1),
                        )
                    st = stagep.tile([P, D], bf16, tag="stage")
                    nc.vector.tensor_copy(st[:rows, :], pt[:rows, :])
                    r0 = min(max(SEG - b * P, 0), rows)   # rows in segment 0
                    if r0 > 0:
                        nc.sync.dma_start(
                            cc_in[0][b * P:b * P + r0,
                                     which * D:(which + 1) * D],
                            st[:r0, :])
                    if r0 < rows:
                        nc.sync.dma_start(
                            cc_in[1][b * P - SEG + r0:b * P - SEG + rows,
                                     which * D:(which + 1) * D],
                            st[r0:rows, :])

            for q in range(2):
                nc.gpsimd.collective_compute(
                    "AllGather", mybir.AluOpType.bypass,
                    replica_groups=[list(range(NC))],
                    ins=[cc_in[q][:]], outs=[cc_out[q][:]],
                )

            # ================= phase B: fused SpMM + epilogues =============
            def selu_into(dst_ap, psum_ap):
                """dst = selu(psum), mostly on ScalarE.

                e2 = exp(x + ln(la));  e3 = relu(la - e2);  r = relu(l*x)
                selu = r - e3
                """
                e2 = tmpp.tile([P, D], f32, tag="tmpd")
                nc.scalar.activation(e2[:], psum_ap,
                                     mybir.ActivationFunctionType.Exp,
                                     bias=ln_la[:])
                e3 = tmpp.tile([P, D], f32, tag="tmpd2")
                nc.scalar.activation(e3[:], e2[:],
                                     mybir.ActivationFunctionType.Relu,
                                     bias=la_c[:], scale=-1.0)
                r = tmpp.tile([P, D], f32, tag="tmpd3")
                nc.scalar.activation(r[:], psum_ap,
                                     mybir.ActivationFunctionType.Relu,
                                     scale=SELU_L)
                nc.vector.tensor_tensor(dst_ap, r[:], e3[:],
                                        mybir.AluOpType.subtract)

            gtile = {}

            def get_gtile(h, g):
                if (h, g) not in gtile:
                    s = strm_base[h][g]
                    n = CBH[g][h]
                    gt = bigp.tile([P, max_grp, W2], bf16, tag="big")
                    if n > 0:
                        nidx = n * P
                        nc.gpsimd.dma_gather(
                            gt[:, 0:n, :], cc_out[h][:],
                            idx_t[h][:, s * 8:(s + n) * 8],
                            num_idxs=nidx, num_idxs_reg=nidx, elem_size=W2,
                            single_packet=False,
                            queue_num=(h * 2 + g % 2),
                        )
                    gtile[(h, g)] = (gt, s)
                return gtile[(h, g)]

            for b in range(NB):
                rows = min(P, NSH - b * P)
                nch = CBH[b][0] + CBH[b][1]
                oht = ohp.tile([P, max_nch_b, P], bf16, tag="oh")
                nc.sync.dma_start(oht[:, 0:nch, :],
                                  oh[:, ohbase[b]:ohbase[b] + nch, :])
                pt = pa.tile([P, W2], f32, space="PSUM", tag="pa")
                nmm = 0
                tot = CBH[b][0] + CBH[b][1]
                if tot == 0:
                    nc.vector.memset(pt[:], 0.0)
                for h in range(2):
                    g = b
                    for j in range(CBH[b][h]):
                        gt, s = get_gtile(h, g)
                        c = strm_base[h][b] + j - s
                        nc.tensor.matmul(
                            pt[:], lhsT=oht[:, h * CBH[b][0] + j, :],
                            rhs=gt[:, c, :],
                            start=(nmm == 0), stop=(nmm == tot - 1))
                        nmm += 1

                # epilogue
                g1b = tmpp.tile([P, D], f32, tag="g1b")
                selu_into(g1b[:], pt[:, 0:D])
                aug = tmpp.tile([P, D], f32, tag="aug")
                selu_into(aug[:], pt[:, D:W2])
                # con partials
                nc.vector.tensor_tensor(accs[:, 0:D], accs[:, 0:D], aug[:],
                                        mybir.AluOpType.add)
                pr = tmpp.tile([P, D], f32, tag="pr")
                nc.vector.tensor_tensor(pr[:], aug[:], g1b[:],
                                        mybir.AluOpType.mult)
                nc.vector.tensor_tensor(accs[:, D:W2], accs[:, D:W2], pr[:],
                                        mybir.AluOpType.add)
                # transposes into gcn1T
                for t in range(DT):
                    ptr = pb.tile([P, P], f32, space="PSUM", tag="pb")
                    nc.tensor.transpose(ptr[:], g1b[:, t * P:(t + 1) * P],
                                        ident[:])
                    nc.vector.tensor_copy(gcn1T[:, t, b * P:(b + 1) * P], ptr[:])
                # logits -> softmax -> S -> DRAM
                pl = pb.tile([P, K], f32, space="PSUM", tag="pb")
                for t in range(DT):
                    nc.tensor.matmul(pl[:], lhsT=gcn1T[:, t, b * P:(b + 1) * P],
                                     rhs=wtt_t[:, t, :],
                                     start=(t == 0), stop=(t == DT - 1))
                nmx = smallp.tile([P, 1], f32, tag="nmx")
                nc.vector.reduce_max(nmx[:], pl[:], axis=mybir.AxisListType.X,
                                     negate=True)
                ex = smallp.tile([P, K], f32, tag="ex")
                sm = smallp.tile([P, 1], f32, tag="sm")
                nc.scalar.activation(ex[:], pl[:],
                                     mybir.ActivationFunctionType.Exp,
                                     bias=nmx[:], accum_out=sm[:])
                rc = smallp.tile([P, 1], f32, tag="rc")
                nc.vector.reciprocal(rc[:], sm[:])
                sb = stagep.tile([P, K], f32, tag="sstage")
                nc.scalar.mul(sb[:], ex[:], rc[:])
                nc.sync.dma_start(s_out_d[b * P:(b + 1) * P, :], sb[:])

            # ================= phase E: stats =================
            for t in range(DT):
                nmt = smallp.tile([P, 1], f32, tag="nmt")
                nc.vector.reduce_max(nmt[:], gcn1T[:, t, 0:NSH],
                                     axis=mybir.AxisListType.X, negate=True)
                nc.scalar.mul(stats_p[:, t:t + 1], nmt[:], -1.0)
                exb = bigp.tile([P, NSH], bf16, tag="big")
                nc.scalar.activation(exb[:, 0:NSH], gcn1T[:, t, 0:NSH],
                                     mybir.ActivationFunctionType.Exp,
                                     bias=nmt[:],
                                     accum_out=stats_p[:, DT + t:DT + t + 1])
            ones = persist.tile([P, 1], f32, tag="ones")
            nc.vector.memset(ones[:], 1.0)
            pv = pa.tile([P, VLEN], f32, space="PSUM", tag="pa")
            nc.tensor.matmul(pv[0:1, :], lhsT=ones[:], rhs=accs[:],
                             start=True, stop=True)
            sv = svp.tile([1, VLEN], f32, tag="sv")
            nc.vector.tensor_copy(sv[:], pv[0:1, :])
            nc.sync.dma_start(stats_v_d[:], sv[:])
            nc.sync.dma_start(stats_p_d[:], stats_p[:])

    nc.compile()
    return nc


# --------------------------------------------------------------------------
# host-side combine of per-core partials
# --------------------------------------------------------------------------

def combine(results, cfg, graph_row, graph_col, gn_vals):
    N, D, K = cfg["N"], cfg["D"], cfg["K"]
    NSH = N // NC
    DT = D // P
    E = float(graph_row.shape[0])

    m, s = [], []
    colsum_aug = np.zeros(D)
    dot = 0.0
    S_full = np.zeros((N, K))
    for c in range(NC):
        sp = np.asarray(results[c]["stats_p"], dtype=np.float64)
        svv = np.asarray(results[c]["stats_v"], dtype=np.float64).reshape(-1)
        m.append(np.concatenate([sp[:, t] for t in range(DT)]))
        s.append(np.concatenate([sp[:, DT + t] for t in range(DT)]))
        colsum_aug += svv[0:D]
        dot += svv[D:2 * D].sum()
        S_full[c * NSH:(c + 1) * NSH] = \
            np.asarray(results[c]["s_out"], dtype=np.float64)[:NSH]
    m = np.stack(m)
    s = np.stack(s)
    M = m.max(axis=0)
    Sg = (np.exp(m - M) * s).sum(axis=0)
    logZ = M + np.log(Sg)

    row = np.asarray(graph_row).astype(np.int64)
    col = np.asarray(graph_col).astype(np.int64)
    deg = np.bincount(col, minlength=N).astype(np.float64)

    trace_gp = np.einsum('ek,ek->', S_full[row], S_full[col])
    nl = S_full.T @ deg
    clsz = S_full.sum(axis=0)

    spectral = -(trace_gp - (nl ** 2).sum() / (2.0 * E)) / (2.0 * E)
    cluster = (np.linalg.norm(clsz) / N * math.sqrt(K) - 1.0) * CLUSTER_REG
    con = -(dot - (logZ * colsum_aug).sum()) / D
    return spectral + cluster + CON_REG * con


# --------------------------------------------------------------------------
# entry point
# --------------------------------------------------------------------------

_BUILD_CACHE = {}


def kernel(features, aug_features, graph_row, graph_col, graph_vals, gn_vals,
           lbl, dense_graph, W1, b1, Wt, bt, _cfg=None, _trace=False):
    cfg = _cfg or FULL
    in_maps, meta = prep(features, aug_features, graph_row, graph_col,
                         gn_vals, W1, Wt, cfg)
    key = tuple(sorted((k, str(v)) for k, v in meta.items()))
    if key not in _BUILD_CACHE:
        _BUILD_CACHE[key] = build(meta)
    nc = _BUILD_CACHE[key]
    res = bass_utils.run_bass_kernel_spmd(nc, in_maps, core_ids=list(range(NC)),
                                          trace=_trace)
    loss = combine(res.results, cfg, graph_row, graph_col, gn_vals)
    out = np.array(loss, dtype=np.float32)
    if _trace:
        return out, res
    return out
